# revision 10
# baseline (speedup 1.0000x reference)
"""Trainium2 Bass kernel for nn_ItemAgg (GNN message passing), v2.

Strategy: shard edges by destination user across 8 cores (zero cross-core
communication).  Per core, users are split into 98 blocks of 128; block edges
are grouped by item PARITY (even items first), each parity group padded to a
global HNT subtiles of 128 edges (NT = 2*HNT slots per block).

Key device structure per block:
  - ONE batched dma_gather (transpose=True) pulls per-edge h1 pre-activations
    from a host-prepared pair-item table P2[item>>1] -> feature-major bf16
    tile GP [128, NT*128]: partitions 0:64 = even-item h1a, 64:128 = odd.
    A second dma_gather pulls per-edge user attention contributions
    q = user_feat @ att_w1[64:] + b_at1 from a per-core table (int16 local
    user ids) -> GQ.
  - Rating one-hots (host planes, 5 rows) are DMA'd into GP's unused pad
    partitions; a single zero-padded assembly matmul per parity then computes
    h1 = Id*h1a + T5*onehot(rating), pairing even/odd subtiles into
    2-lane [128, N] tiles (partitions 0:64 = even-subtile edges, 64:128 = odd).
  - MLP chain with block-diagonal weights at [128, 512] granularity; relu+bias
    on ScalarE; attention logits via per-pair N=2 matmuls; exp batched.
  - Scatter-sum to users via per-subtile one-hot matmuls (DVE is_equal builds,
    batched 4 subtiles per op); softmax numerator/denominator accumulated in
    one PSUM bank per block, then normalize + final Linear + DMA out.

Softmax is computed without max subtraction (logits are O(0.1); exp is safe).
"""

import os
import sys

import numpy as np
import ml_dtypes

sys.path.insert(0, "/opt/trn_rl_repo")

import concourse.bass as bass
import concourse.bacc as bacc
import concourse.mybir as mybir
import concourse.tile as tile
from concourse.bass_utils import run_bass_kernel_spmd
from concourse.masks import make_identity

U, I, E, D, R = 100000, 50000, 2000000, 64, 5
NCORES = 8
UPC = U // NCORES            # users per core
NBLK = (UPC + 127) // 128    # 128-user blocks per core
QROWS = NBLK * 128           # padded per-core q-table rows (12544)
BF16 = mybir.dt.bfloat16
F32 = mybir.dt.float32
I16 = mybir.dt.int16
NB = np.dtype(ml_dtypes.bfloat16)


def _host_shard(row_idxs, col_idxs, rating):
    """Per-core, per-block, parity-grouped edge slots + index planes."""
    row_idxs = np.asarray(row_idxs, dtype=np.int64)
    col_idxs = np.asarray(col_idxs, dtype=np.int64)
    rating = np.asarray(rating, dtype=np.int64)

    per_core = []
    hmax = 1
    for c in range(NCORES):
        base = c * UPC
        sel = (col_idxs >= base) & (col_idxs < base + UPC)
        it = row_idxs[sel]
        rt = rating[sel]
        loc = col_idxs[sel] - base
        # sort by (block, parity, uid)
        blk = loc >> 7
        par = it & 1
        order = np.lexsort((loc, par, blk))
        it, rt, loc, blk, par = it[order], rt[order], loc[order], blk[order], par[order]
        # per (block, parity) counts
        key = blk * 2 + par
        cnt = np.bincount(key, minlength=NBLK * 2)
        hmax = max(hmax, int(((cnt + 127) // 128).max()))
        per_core.append((it, rt, loc, cnt))

    HNT = hmax
    NT = 2 * HNT
    SLOTS = NT * 128
    HS = HNT * 128

    shards = []
    for c in range(NCORES):
        it, rt, loc, cnt = per_core[c]
        starts = np.concatenate(([0], np.cumsum(cnt)))
        itP = np.zeros((NBLK, SLOTS), dtype=np.int16)
        uQ = np.full((NBLK, SLOTS), QROWS - 1, dtype=np.int16)
        rl = np.full((NBLK, 128, NT), 300.0, dtype=np.float32)
        orT = np.zeros((NBLK, 5, SLOTS), dtype=NB)
        for b in range(NBLK):
            for par in range(2):
                k = b * 2 + par
                s, n = starts[k], cnt[k]
                off = par * HS
                sl = np.arange(off, off + n)
                itP[b, sl] = (it[s : s + n] >> 1).astype(np.int16)
                uQ[b, sl] = loc[s : s + n].astype(np.int16)  # local uid in core
                # rl is per (partition, subtile): slot -> (p = sl%128, t = sl//128)
                rl[b, sl % 128, sl // 128] = (loc[s : s + n] & 127).astype(np.float32)
                orT[b, rt[s : s + n], sl] = 1.0
        # pack idx planes: plane[p, col] = idx[col*16 + p%16], replicated x8
        def pack(ix):
            # ix [NBLK, SLOTS] -> [NBLK, 128, SLOTS//16]
            pl = ix.reshape(NBLK, SLOTS // 16, 16).transpose(0, 2, 1)
            return np.ascontiguousarray(np.tile(pl, (1, 8, 1)))
        shards.append(
            dict(
                idxP=pack(itP),
                rlT=np.ascontiguousarray(
                    rl.transpose(0, 2, 1).reshape(NBLK, 1, SLOTS)).astype(NB),
                rl=np.ascontiguousarray(rl).astype(NB),
                orTe=np.ascontiguousarray(orT[:, :, :HS]),
                orTo=np.ascontiguousarray(orT[:, :, HS:]),
            )
        )
    return HNT, shards


def _build_program(HNT):
    NT = 2 * HNT
    SLOTS = NT * 128
    HS = HNT * 128
    nc = bacc.Bacc(None, target_bir_lowering=False, debug=False)

    # --- I/O ---
    p2_t = nc.declare_dram_parameter("p2_tab", [I // 2, 128], BF16, isOutput=False)
    q1_t = nc.declare_dram_parameter("q1_tab", [QROWS, 128], BF16, isOutput=False)
    q2_t = nc.declare_dram_parameter("q2_tab", [QROWS, 128], BF16, isOutput=False)
    idxP = nc.declare_dram_parameter("idxP", [NBLK, 128, SLOTS // 16], I16, isOutput=False)
    rl_t = nc.declare_dram_parameter("rl", [NBLK, 128, NT], BF16, isOutput=False)
    rlT_t = nc.declare_dram_parameter("rlT", [NBLK, 1, SLOTS], BF16, isOutput=False)
    ones_r = nc.declare_dram_parameter("ones_r", [1, 128], BF16, isOutput=False)
    iotap = nc.declare_dram_parameter("iotap", [128, 1], F32, isOutput=False)
    iota4 = nc.declare_dram_parameter("iota4", [128, 512], BF16, isOutput=False)
    orTe = nc.declare_dram_parameter("orTe", [NBLK, 5, HS], BF16, isOutput=False)
    orTo = nc.declare_dram_parameter("orTo", [NBLK, 5, HS], BF16, isOutput=False)
    lhsE2 = nc.declare_dram_parameter("lhsE2", [128, 128], BF16, isOutput=False)
    lhsO2 = nc.declare_dram_parameter("lhsO2", [128, 128], BF16, isOutput=False)
    wgv2 = nc.declare_dram_parameter("wgv2bd", [128, 128], BF16, isOutput=False)
    wat1 = nc.declare_dram_parameter("wat1bd", [128, 128], BF16, isOutput=False)
    wat2 = nc.declare_dram_parameter("wat2bd", [128, 128], BF16, isOutput=False)
    w3bd = nc.declare_dram_parameter("w3bd", [128, 2], BF16, isOutput=False)
    idtop = nc.declare_dram_parameter("idtop", [64, 128], BF16, isOutput=False)
    idbot = nc.declare_dram_parameter("idbot", [64, 128], BF16, isOutput=False)
    bgv1 = nc.declare_dram_parameter("bgv1", [128, 1], F32, isOutput=False)
    bgv2 = nc.declare_dram_parameter("bgv2", [128, 1], F32, isOutput=False)
    bat2 = nc.declare_dram_parameter("bat2", [128, 1], F32, isOutput=False)
    b3c = nc.declare_dram_parameter("b3c", [128, 1], F32, isOutput=False)
    wout = nc.declare_dram_parameter("wout", [D, D], BF16, isOutput=False)
    wb_t = nc.declare_dram_parameter("wb_t", [128, D], F32, isOutput=False)
    iota = nc.declare_dram_parameter("iota_r", [128, 128], F32, isOutput=False)
    out = nc.declare_dram_parameter("out", [NBLK * 128, D], F32, isOutput=True)

    # pair groups
    groups = []
    g0 = 0
    while g0 < HNT:
        gw = min(4, HNT - g0)
        groups.append((g0, gw))
        g0 += gw

    with tile.TileContext(nc) as tc:
        with (
            tc.tile_pool(name="const", bufs=1) as cp,
            tc.tile_pool(name="idx", bufs=3) as ip,
            tc.tile_pool(name="gath", bufs=3) as gp,
            tc.tile_pool(name="mlp", bufs=3) as wp,
            tc.tile_pool(name="sc", bufs=3) as sp,
            tc.tile_pool(name="fin", bufs=2) as fp,
            tc.tile_pool(name="pmlp", bufs=2, space="PSUM") as pm,
            tc.tile_pool(name="pxt", bufs=2, space="PSUM") as pt,
            tc.tile_pool(name="pwl", bufs=2, space="PSUM") as pw,
            tc.tile_pool(name="pacc", bufs=2, space="PSUM") as pa,
        ):
            id_f = cp.tile([128, 128], F32, tag="id_f")
            make_identity(nc, id_f[:])
            id_b = cp.tile([128, 128], BF16, tag="id_b")
            nc.vector.tensor_copy(id_b[:], id_f[:])
            c_iota = cp.tile([128, 128], F32, tag="c_iota")
            nc.sync.dma_start(c_iota[:], iota[:])

            def ld(name, shape, dt, src):
                t = cp.tile(shape, dt, tag=name)
                nc.sync.dma_start(t[:], src[:])
                return t

            c_lE = ld("c_lE", [128, 128], BF16, lhsE2)
            c_lO = ld("c_lO", [128, 128], BF16, lhsO2)
            c_gv2 = ld("c_gv2", [128, 128], BF16, wgv2)
            c_at1 = ld("c_at1", [128, 128], BF16, wat1)
            c_at2 = ld("c_at2", [128, 128], BF16, wat2)
            c_w3 = ld("c_w3", [128, 2], BF16, w3bd)
            c_idt = ld("c_idt", [64, 128], BF16, idtop)
            c_idb = ld("c_idb", [64, 128], BF16, idbot)
            c_bgv1 = ld("c_bgv1", [128, 1], F32, bgv1)
            c_bgv2 = ld("c_bgv2", [128, 1], F32, bgv2)
            c_bat2 = ld("c_bat2", [128, 1], F32, bat2)
            c_b3 = ld("c_b3", [128, 1], F32, b3c)
            c_wout = ld("c_wout", [D, D], BF16, wout)
            c_ones = ld("c_ones", [1, 128], BF16, ones_r)
            c_iop = ld("c_iop", [128, 1], F32, iotap)
            c_io4 = ld("c_io4", [128, 512], BF16, iota4)
            c_wb = ld("c_wb", [128, D], F32, wb_t)

            for b in range(NBLK):
                t_iP = ip.tile([128, SLOTS // 16], I16, tag="t_iP")
                nc.sync.dma_start(t_iP[:], idxP[b])
                t_rl = ip.tile([128, NT], BF16, tag="t_rl")
                nc.sync.dma_start(t_rl[:], rl_t[b])
                t_rlT = ip.tile([1, SLOTS], BF16, tag="t_rlT")
                nc.sync.dma_start(t_rlT[:], rlT_t[b])

                GP = gp.tile([128, SLOTS], BF16, tag="GP")
                nc.gpsimd.dma_gather(
                    out_ap=GP[:].rearrange("p (a n) -> p a n", a=1),
                    in_ap=p2_t[:],
                    idxs_ap=t_iP[:],
                    num_idxs=SLOTS,
                    num_idxs_reg=SLOTS,
                    elem_size=128,
                    transpose=True,
                    single_packet=False,
                )
                nc.sync.dma_start(GP[64:69, 0:HS], orTe[b])
                nc.sync.dma_start(GP[0:5, HS:SLOTS], orTo[b])
                Qb = gp.tile([128, 128], BF16, tag="GQ")
                nc.sync.dma_start(Qb[:], q1_t[b * 128 : (b + 1) * 128, :])
                Qb2 = gp.tile([128, 128], BF16, tag="GQ2")
                nc.sync.dma_start(Qb2[:], q2_t[b * 128 : (b + 1) * 128, :])

                acc = pa.tile([128, 512], F32, tag="acc")
                nsc = 2 * HNT  # scatter matmul count per block
                isc = 0

                for (g0, gw) in groups:
                    N = gw * 128
                    se = slice(g0 * 128, g0 * 128 + N)
                    so = slice(HS + g0 * 128, HS + g0 * 128 + N)

                    rbE = pm.tile([128, 512], F32, tag="mlpp")
                    nc.tensor.matmul(rbE[:, 0:N], c_ones[:], t_rlT[:, se], start=True, stop=True)
                    StE = sp.tile([128, 512], BF16, tag="StE")
                    nc.vector.tensor_tensor(
                        StE[:, 0:N], rbE[:, 0:N], c_iop[:].to_broadcast([128, N]),
                        mybir.AluOpType.is_equal,
                    )
                    rbO = pm.tile([128, 512], F32, tag="mlpp")
                    nc.tensor.matmul(rbO[:, 0:N], c_ones[:], t_rlT[:, so], start=True, stop=True)
                    StO = sp.tile([128, 512], BF16, tag="StO")
                    nc.vector.tensor_tensor(
                        StO[:, 0:N], rbO[:, 0:N], c_iop[:].to_broadcast([128, N]),
                        mybir.AluOpType.is_equal,
                    )
                    h1p = pm.tile([128, 512], F32, tag="mlpp")
                    nc.tensor.matmul(h1p[:, 0:N], c_lE[:], GP[:, se], start=True, stop=False)
                    nc.tensor.matmul(h1p[:, 0:N], c_lO[:], GP[:, so], start=False, stop=True)
                    h1s = wp.tile([128, 512], BF16, tag="h1s")
                    nc.scalar.activation(
                        h1s[:, 0:N], h1p[:, 0:N],
                        mybir.ActivationFunctionType.Relu, bias=c_bgv1[:],
                    )
                    xp = pm.tile([128, 512], F32, tag="mlpp")
                    nc.tensor.matmul(xp[:, 0:N], c_gv2[:], h1s[:, 0:N], start=True, stop=True)
                    AUx = wp.tile([128, 512], BF16, tag="AUx")
                    nc.scalar.activation(
                        AUx[:, 0:N], xp[:, 0:N],
                        mybir.ActivationFunctionType.Relu, bias=c_bgv2[:],
                    )
                    a1p = pm.tile([128, 512], F32, tag="mlpp")
                    nc.tensor.matmul(a1p[:, 0:N], c_at1[:], AUx[:, 0:N], start=True, stop=False)
                    nc.tensor.matmul(a1p[:, 0:N], Qb[:], StE[:, 0:N], start=False, stop=False)
                    nc.tensor.matmul(a1p[:, 0:N], Qb2[:], StO[:, 0:N], start=False, stop=True)
                    a1s = wp.tile([128, 512], BF16, tag="a1s")
                    nc.scalar.activation(
                        a1s[:, 0:N], a1p[:, 0:N], mybir.ActivationFunctionType.Relu,
                    )
                    a2p = pm.tile([128, 512], F32, tag="mlpp")
                    nc.tensor.matmul(a2p[:, 0:N], c_at2[:], a1s[:, 0:N], start=True, stop=True)
                    a2s = wp.tile([128, 512], BF16, tag="a2s")
                    nc.scalar.activation(
                        a2s[:, 0:N], a2p[:, 0:N],
                        mybir.ActivationFunctionType.Relu, bias=c_bat2[:],
                    )

                    wlP = pw.tile([128, 8], F32, tag="wlP")
                    xtP = pt.tile([128, 512], BF16, tag="xtP")
                    for k in range(gw):
                        ks = slice(k * 128, (k + 1) * 128)
                        nc.tensor.matmul(
                            wlP[:, 2 * k : 2 * k + 2], a2s[:, ks], c_w3[:],
                            start=True, stop=True,
                        )
                        nc.tensor.transpose(xtP[:, ks], AUx[:, ks], id_b[:])

                    Pg = sp.tile([128, 8], BF16, tag="Pg")
                    nc.scalar.activation(
                        Pg[:, 0 : 2 * gw], wlP[:, 0 : 2 * gw],
                        mybir.ActivationFunctionType.Exp, bias=c_b3[:],
                    )

                    rs = sp.tile([128, 4 * 130], BF16, tag="rs")
                    nc.vector.tensor_tensor(
                        rs[:, 0 : gw * 130].rearrange("p (k h c) -> p k h c", h=2, c=65)[:, :, :, 0:64],
                        xtP[:, 0:N].rearrange("p (k h c) -> p k h c", h=2, c=64),
                        Pg[:, 0 : 2 * gw].rearrange("p (k h) -> p k h", h=2)
                        .to_broadcast([128, gw, 2, 64]),
                        mybir.AluOpType.mult,
                    )
                    nc.vector.tensor_copy(
                        rs[:, 0 : gw * 130].rearrange("p (k h c) -> p k h c", h=2, c=65)[:, :, :, 64:65],
                        Pg[:, 0 : 2 * gw].rearrange("p (k h a) -> p k h a", h=2, a=1),
                    )

                    Se = sp.tile([128, 512], BF16, tag="Se")
                    nc.vector.tensor_tensor(
                        Se[:, 0:N], c_io4[:, 0:N],
                        t_rl[:, g0 : g0 + gw].to_broadcast([128, gw, 128]),
                        mybir.AluOpType.is_equal,
                    )
                    So = sp.tile([128, 512], BF16, tag="So")
                    nc.vector.tensor_tensor(
                        So[:, 0:N], c_io4[:, 0:N],
                        t_rl[:, HNT + g0 : HNT + g0 + gw].to_broadcast([128, gw, 128]),
                        mybir.AluOpType.is_equal,
                    )

                    for k in range(gw):
                        ks = slice(k * 128, (k + 1) * 128)
                        nc.tensor.matmul(
                            acc[:, 0:65], Se[:, ks], rs[:, k * 130 : k * 130 + 65],
                            start=(isc == 0), stop=False,
                        )
                        isc += 1
                        nc.tensor.matmul(
                            acc[:, 0:65], So[:, ks], rs[:, k * 130 + 65 : k * 130 + 130],
                            start=False, stop=(isc == nsc - 1),
                        )
                        isc += 1

                # block finalize
                s_eps = fp.tile([128, 1], F32, tag="s_eps")
                nc.vector.tensor_scalar_add(s_eps[:], acc[:, 64:65], 1e-30)
                rcp = fp.tile([128, 1], F32, tag="rcp")
                nc.vector.reciprocal(rcp[:], s_eps[:])
                hn = fp.tile([128, D], BF16, tag="hn")
                nc.vector.tensor_tensor(
                    hn[:], acc[:, 0:64], rcp[:].to_broadcast([128, D]),
                    mybir.AluOpType.mult,
                )
                xtF = pt.tile([128, 512], BF16, tag="xtP")
                nc.tensor.transpose(xtF[0:64, 0:128], hn[:], id_b[:])
                hts = fp.tile([D, 128], BF16, tag="hts")
                nc.scalar.copy(hts[:], xtF[0:64, 0:128])
                nc.tensor.matmul(
                    acc[:, 256:320], hts[:], c_wout[:], start=True, stop=True
                )
                outs = fp.tile([128, D], F32, tag="outs")
                nc.vector.tensor_tensor(
                    outs[:], acc[:, 256:320], c_wb[:], mybir.AluOpType.add
                )
                nc.sync.dma_start(out[b * 128 : (b + 1) * 128, :], outs[:])

    nc.compile()
    return nc


def kernel(**inputs):
    rowi = np.asarray(inputs["row_idxs"])
    coli = np.asarray(inputs["col_idxs"])
    rati = np.asarray(inputs["rating"])
    HNT, shards = _host_shard(rowi, coli, rati)

    nc = _build_program(HNT)

    def f32(x):
        return np.ascontiguousarray(np.asarray(x, dtype=np.float32))

    item_feat = f32(inputs["item_feat"])
    user_feat = f32(inputs["user_feat"])
    rating_feat = f32(inputs["rating_feat"])
    gv_w1 = f32(inputs["gv_w1"])
    att_w1 = f32(inputs["att_w1"])

    # pair-item table: h1 item contribution
    h1a = item_feat @ gv_w1[:D]                      # [I, 64]
    p2 = np.zeros((I // 2, 128), dtype=NB)
    p2[:, 0:64] = h1a[0::2]
    p2[:, 64:128] = h1a[1::2]

    T5 = rating_feat @ gv_w1[D:]                     # [5, 64]
    I64 = np.eye(64, dtype=np.float32)
    lE = np.zeros((128, 128), dtype=NB)
    lE[0:64, 0:64] = I64
    lE[64:69, 0:64] = T5
    lO = np.zeros((128, 128), dtype=NB)
    lO[64:128, 64:128] = I64
    lO[0:5, 64:128] = T5

    def bd(w):
        m = np.zeros((128, 128), dtype=NB)
        m[0:64, 0:64] = w
        m[64:128, 64:128] = w
        return m

    w3 = f32(inputs["att_w3"]).reshape(64, 1)
    w3bd = np.zeros((128, 2), dtype=NB)
    w3bd[0:64, 0:1] = w3
    w3bd[64:128, 1:2] = w3
    idt = np.zeros((64, 128), dtype=NB)
    idt[:, 0:64] = I64
    idb = np.zeros((64, 128), dtype=NB)
    idb[:, 64:128] = I64

    def lane2(v):
        return np.concatenate([v.reshape(-1), v.reshape(-1)]).reshape(128, 1).astype(np.float32)

    common = dict(
        p2_tab=p2,
        idxP=None, idxQ=None, rl=None, orTe=None, orTo=None, q1_tab=None,
        lhsE2=lE, lhsO2=lO,
        wgv2bd=bd(f32(inputs["gv_w2"])),
        wat1bd=bd(att_w1[:D]),
        wat2bd=bd(f32(inputs["att_w2"])),
        w3bd=w3bd, idtop=idt, idbot=idb,
        bgv1=lane2(f32(inputs["gv_b1"])),
        bgv2=lane2(f32(inputs["gv_b2"])),
        bat2=lane2(f32(inputs["att_b2"])),
        b3c=np.full((128, 1), np.float32(np.asarray(inputs["att_b3"]).reshape(-1)[0]),
                    dtype=np.float32),
        wout=f32(inputs["w_w"]).astype(NB),
        wb_t=np.tile(f32(inputs["w_b"]).reshape(1, D), (128, 1)),
        iota_r=np.tile(np.arange(128, dtype=np.float32), (128, 1)),
        ones_r=np.ones((1, 128), dtype=NB),
        iotap=np.arange(128, dtype=np.float32).reshape(128, 1),
        iota4=np.tile(np.arange(128, dtype=np.float32), (128, 4)).astype(NB),
    )

    att_b1 = f32(inputs["att_b1"]).reshape(1, D)
    in_maps = []
    for c in range(NCORES):
        base = c * UPC
        q1 = np.zeros((QROWS, 128), dtype=NB)
        q1[0:UPC, 0:64] = user_feat[base : base + UPC] @ att_w1[D:] + att_b1
        m = dict(common)
        m["q1_tab"] = q1
        q2 = np.zeros((QROWS, 128), dtype=NB)
        q2[:, 64:128] = q1[:, 0:64]
        m["q2_tab"] = q2
        m["idxP"] = shards[c]["idxP"]
        m["rlT"] = shards[c]["rlT"]
        m["rl"] = shards[c]["rl"]
        m["orTe"] = shards[c]["orTe"]
        m["orTo"] = shards[c]["orTo"]
        in_maps.append(m)

    trace = os.environ.get("ITEMAGG_TRACE") == "1"
    res = run_bass_kernel_spmd(nc, in_maps, list(range(NCORES)), trace=trace)
    global LAST_RESULT
    LAST_RESULT = res
    outs = [res.results[c]["out"][:UPC] for c in range(NCORES)]
    return np.concatenate(outs, axis=0).astype(np.float32)


LAST_RESULT = None

if __name__ == "__main__":
    pass


# revision 11
# speedup vs baseline: 1.1813x; 1.1813x over previous
"""Trainium2 Bass kernel for nn_ItemAgg (GNN message passing), v2.

Strategy: shard edges by destination user across 8 cores (zero cross-core
communication).  Per core, users are split into 98 blocks of 128; block edges
are grouped by item PARITY (even items first), each parity group padded to a
global HNT subtiles of 128 edges (NT = 2*HNT slots per block).

Key device structure per block:
  - ONE batched dma_gather (transpose=True) pulls per-edge h1 pre-activations
    from a host-prepared pair-item table P2[item>>1] -> feature-major bf16
    tile GP [128, NT*128]: partitions 0:64 = even-item h1a, 64:128 = odd.
    A second dma_gather pulls per-edge user attention contributions
    q = user_feat @ att_w1[64:] + b_at1 from a per-core table (int16 local
    user ids) -> GQ.
  - Rating one-hots (host planes, 5 rows) are DMA'd into GP's unused pad
    partitions; a single zero-padded assembly matmul per parity then computes
    h1 = Id*h1a + T5*onehot(rating), pairing even/odd subtiles into
    2-lane [128, N] tiles (partitions 0:64 = even-subtile edges, 64:128 = odd).
  - MLP chain with block-diagonal weights at [128, 512] granularity; relu+bias
    on ScalarE; attention logits via per-pair N=2 matmuls; exp batched.
  - Scatter-sum to users via per-subtile one-hot matmuls (DVE is_equal builds,
    batched 4 subtiles per op); softmax numerator/denominator accumulated in
    one PSUM bank per block, then normalize + final Linear + DMA out.

Softmax is computed without max subtraction (logits are O(0.1); exp is safe).
"""

import os
import sys

import numpy as np
import ml_dtypes

sys.path.insert(0, "/opt/trn_rl_repo")

import concourse.bass as bass
import concourse.bacc as bacc
import concourse.mybir as mybir
import concourse.tile as tile
from concourse.bass_utils import run_bass_kernel_spmd
from concourse.masks import make_identity

U, I, E, D, R = 100000, 50000, 2000000, 64, 5
NCORES = 8
UPC = U // NCORES            # users per core
NBLK = (UPC + 127) // 128    # 128-user blocks per core
QROWS = NBLK * 128           # padded per-core q-table rows (12544)
BF16 = mybir.dt.bfloat16
F32 = mybir.dt.float32
I16 = mybir.dt.int16
NB = np.dtype(ml_dtypes.bfloat16)


def _host_shard(row_idxs, col_idxs, rating):
    """Per-core, per-block, parity-grouped edge slots + index planes."""
    row_idxs = np.asarray(row_idxs, dtype=np.int64)
    col_idxs = np.asarray(col_idxs, dtype=np.int64)
    rating = np.asarray(rating, dtype=np.int64)

    per_core = []
    hmax = 1
    for c in range(NCORES):
        base = c * UPC
        sel = (col_idxs >= base) & (col_idxs < base + UPC)
        it = row_idxs[sel]
        rt = rating[sel]
        loc = col_idxs[sel] - base
        # sort by (block, parity, uid)
        blk = loc >> 7
        par = it & 1
        order = np.lexsort((loc, par, blk))
        it, rt, loc, blk, par = it[order], rt[order], loc[order], blk[order], par[order]
        # per (block, parity) counts
        key = blk * 2 + par
        cnt = np.bincount(key, minlength=NBLK * 2)
        hmax = max(hmax, int(((cnt + 127) // 128).max()))
        per_core.append((it, rt, loc, cnt))

    HNT = hmax
    NT = 2 * HNT
    SLOTS = NT * 128
    HS = HNT * 128

    shards = []
    for c in range(NCORES):
        it, rt, loc, cnt = per_core[c]
        starts = np.concatenate(([0], np.cumsum(cnt)))
        itP = np.zeros((NBLK, SLOTS), dtype=np.int16)
        uQ = np.full((NBLK, SLOTS), QROWS - 1, dtype=np.int16)
        rl = np.full((NBLK, 128, NT), 300.0, dtype=np.float32)
        orT = np.zeros((NBLK, 5, SLOTS), dtype=NB)
        for b in range(NBLK):
            for par in range(2):
                k = b * 2 + par
                s, n = starts[k], cnt[k]
                off = par * HS
                sl = np.arange(off, off + n)
                itP[b, sl] = (it[s : s + n] >> 1).astype(np.int16)
                uQ[b, sl] = loc[s : s + n].astype(np.int16)  # local uid in core
                # rl is per (partition, subtile): slot -> (p = sl%128, t = sl//128)
                rl[b, sl % 128, sl // 128] = (loc[s : s + n] & 127).astype(np.float32)
                orT[b, rt[s : s + n], sl] = 1.0
        # pack idx planes: plane[p, col] = idx[col*16 + p%16], replicated x8
        def pack(ix):
            # ix [NBLK, SLOTS] -> [NBLK, 128, SLOTS//16]
            pl = ix.reshape(NBLK, SLOTS // 16, 16).transpose(0, 2, 1)
            return np.ascontiguousarray(np.tile(pl, (1, 8, 1)))
        shards.append(
            dict(
                idxP=pack(itP),
                rlT=np.ascontiguousarray(
                    rl.transpose(0, 2, 1).reshape(NBLK, 1, SLOTS)).astype(NB),
                rl=np.ascontiguousarray(rl),
                orTe=np.ascontiguousarray(orT[:, :, :HS]),
                orTo=np.ascontiguousarray(orT[:, :, HS:]),
            )
        )
    return HNT, shards


def _build_program(HNT):
    NT = 2 * HNT
    SLOTS = NT * 128
    HS = HNT * 128
    nc = bacc.Bacc(None, target_bir_lowering=False, debug=False)

    # --- I/O ---
    p2_t = nc.declare_dram_parameter("p2_tab", [I // 2, 128], BF16, isOutput=False)
    q1_t = nc.declare_dram_parameter("q1_tab", [QROWS, 128], BF16, isOutput=False)
    q2_t = nc.declare_dram_parameter("q2_tab", [QROWS, 128], BF16, isOutput=False)
    idxP = nc.declare_dram_parameter("idxP", [NBLK, 128, SLOTS // 16], I16, isOutput=False)
    rl_t = nc.declare_dram_parameter("rl", [NBLK, 128, NT], F32, isOutput=False)
    rlT_t = nc.declare_dram_parameter("rlT", [NBLK, 1, SLOTS], BF16, isOutput=False)
    ones_r = nc.declare_dram_parameter("ones_r", [1, 128], BF16, isOutput=False)
    iotap = nc.declare_dram_parameter("iotap", [128, 1], F32, isOutput=False)
    iota4 = nc.declare_dram_parameter("iota4", [128, 512], F32, isOutput=False)
    orTe = nc.declare_dram_parameter("orTe", [NBLK, 5, HS], BF16, isOutput=False)
    orTo = nc.declare_dram_parameter("orTo", [NBLK, 5, HS], BF16, isOutput=False)
    lhsE2 = nc.declare_dram_parameter("lhsE2", [128, 128], BF16, isOutput=False)
    lhsO2 = nc.declare_dram_parameter("lhsO2", [128, 128], BF16, isOutput=False)
    wgv2 = nc.declare_dram_parameter("wgv2bd", [128, 128], BF16, isOutput=False)
    wat1 = nc.declare_dram_parameter("wat1bd", [128, 128], BF16, isOutput=False)
    wat2 = nc.declare_dram_parameter("wat2bd", [128, 128], BF16, isOutput=False)
    w3bd = nc.declare_dram_parameter("w3bd", [128, 2], BF16, isOutput=False)
    idtop = nc.declare_dram_parameter("idtop", [64, 128], BF16, isOutput=False)
    idbot = nc.declare_dram_parameter("idbot", [64, 128], BF16, isOutput=False)
    bgv1 = nc.declare_dram_parameter("bgv1", [128, 1], F32, isOutput=False)
    bgv2 = nc.declare_dram_parameter("bgv2", [128, 1], F32, isOutput=False)
    bat2 = nc.declare_dram_parameter("bat2", [128, 1], F32, isOutput=False)
    b3c = nc.declare_dram_parameter("b3c", [128, 1], F32, isOutput=False)
    wout = nc.declare_dram_parameter("wout", [D, D], BF16, isOutput=False)
    wb_t = nc.declare_dram_parameter("wb_t", [128, D], F32, isOutput=False)
    iota = nc.declare_dram_parameter("iota_r", [128, 128], F32, isOutput=False)
    out = nc.declare_dram_parameter("out", [NBLK * 128, D], F32, isOutput=True)

    # pair groups
    groups = []
    g0 = 0
    while g0 < HNT:
        gw = min(4, HNT - g0)
        groups.append((g0, gw))
        g0 += gw

    with tile.TileContext(nc) as tc:
        with (
            tc.tile_pool(name="const", bufs=1) as cp,
            tc.tile_pool(name="idx", bufs=2) as ip,
            tc.tile_pool(name="gath", bufs=2) as gp,
            tc.tile_pool(name="mlp", bufs=2) as wp,
            tc.tile_pool(name="sc", bufs=2) as sp,
            tc.tile_pool(name="fin", bufs=2) as fp,
            tc.tile_pool(name="pmlp", bufs=2, space="PSUM") as pm,
            tc.tile_pool(name="pxt", bufs=2, space="PSUM") as pt,
            tc.tile_pool(name="pwl", bufs=2, space="PSUM") as pw,
            tc.tile_pool(name="pacc", bufs=2, space="PSUM") as pa,
        ):
            id_f = cp.tile([128, 128], F32, tag="id_f")
            make_identity(nc, id_f[:])
            id_b = cp.tile([128, 128], BF16, tag="id_b")
            nc.vector.tensor_copy(id_b[:], id_f[:])
            c_iota = cp.tile([128, 128], F32, tag="c_iota")
            nc.sync.dma_start(c_iota[:], iota[:])

            def ld(name, shape, dt, src):
                t = cp.tile(shape, dt, tag=name)
                nc.sync.dma_start(t[:], src[:])
                return t

            c_lE = ld("c_lE", [128, 128], BF16, lhsE2)
            c_lO = ld("c_lO", [128, 128], BF16, lhsO2)
            c_gv2 = ld("c_gv2", [128, 128], BF16, wgv2)
            c_at1 = ld("c_at1", [128, 128], BF16, wat1)
            c_at2 = ld("c_at2", [128, 128], BF16, wat2)
            c_w3 = ld("c_w3", [128, 2], BF16, w3bd)
            c_idt = ld("c_idt", [64, 128], BF16, idtop)
            c_idb = ld("c_idb", [64, 128], BF16, idbot)
            c_bgv1 = ld("c_bgv1", [128, 1], F32, bgv1)
            c_bgv2 = ld("c_bgv2", [128, 1], F32, bgv2)
            c_bat2 = ld("c_bat2", [128, 1], F32, bat2)
            c_b3 = ld("c_b3", [128, 1], F32, b3c)
            c_wout = ld("c_wout", [D, D], BF16, wout)
            c_ones = ld("c_ones", [1, 128], BF16, ones_r)
            c_iop = ld("c_iop", [128, 1], F32, iotap)
            c_io4 = ld("c_io4", [128, 512], F32, iota4)
            c_wb = ld("c_wb", [128, D], F32, wb_t)

            for b in range(NBLK):
                t_iP = ip.tile([128, SLOTS // 16], I16, tag="t_iP")
                nc.sync.dma_start(t_iP[:], idxP[b])
                t_rl = ip.tile([128, NT], F32, tag="t_rl")
                nc.sync.dma_start(t_rl[:], rl_t[b])
                t_rlT = ip.tile([1, SLOTS], BF16, tag="t_rlT")
                nc.sync.dma_start(t_rlT[:], rlT_t[b])

                GP = gp.tile([128, SLOTS], BF16, tag="GP")
                nc.gpsimd.dma_gather(
                    out_ap=GP[:].rearrange("p (a n) -> p a n", a=1),
                    in_ap=p2_t[:],
                    idxs_ap=t_iP[:],
                    num_idxs=SLOTS,
                    num_idxs_reg=SLOTS,
                    elem_size=128,
                    transpose=True,
                    single_packet=False,
                )
                nc.sync.dma_start(GP[64:69, 0:HS], orTe[b])
                nc.sync.dma_start(GP[0:5, HS:SLOTS], orTo[b])
                Qb = gp.tile([128, 128], BF16, tag="GQ")
                nc.sync.dma_start(Qb[:], q1_t[b * 128 : (b + 1) * 128, :])
                Qb2 = gp.tile([128, 128], BF16, tag="GQ2")
                nc.sync.dma_start(Qb2[:], q2_t[b * 128 : (b + 1) * 128, :])

                acc = pa.tile([128, 512], F32, tag="acc")
                nsc = 2 * HNT  # scatter matmul count per block
                isc = 0

                for (g0, gw) in groups:
                    N = gw * 128
                    se = slice(g0 * 128, g0 * 128 + N)
                    so = slice(HS + g0 * 128, HS + g0 * 128 + N)

                    rbE = pm.tile([128, 512], F32, tag="mlpp")
                    nc.tensor.matmul(rbE[:, 0:N], c_ones[:], t_rlT[:, se], start=True, stop=True)
                    StE = sp.tile([128, 512], BF16, tag="StE")
                    nc.vector.tensor_tensor(
                        StE[:, 0:N], rbE[:, 0:N], c_iop[:].to_broadcast([128, N]),
                        mybir.AluOpType.is_equal,
                    )
                    rbO = pm.tile([128, 512], F32, tag="mlpp")
                    nc.tensor.matmul(rbO[:, 0:N], c_ones[:], t_rlT[:, so], start=True, stop=True)
                    StO = sp.tile([128, 512], BF16, tag="StO")
                    nc.vector.tensor_tensor(
                        StO[:, 0:N], rbO[:, 0:N], c_iop[:].to_broadcast([128, N]),
                        mybir.AluOpType.is_equal,
                    )
                    h1p = pm.tile([128, 512], F32, tag="mlpp")
                    nc.tensor.matmul(h1p[:, 0:N], c_lE[:], GP[:, se], start=True, stop=False)
                    nc.tensor.matmul(h1p[:, 0:N], c_lO[:], GP[:, so], start=False, stop=True)
                    h1s = wp.tile([128, 512], BF16, tag="h1s")
                    nc.scalar.activation(
                        h1s[:, 0:N], h1p[:, 0:N],
                        mybir.ActivationFunctionType.Relu, bias=c_bgv1[:],
                    )
                    xp = pm.tile([128, 512], F32, tag="mlpp")
                    nc.tensor.matmul(xp[:, 0:N], c_gv2[:], h1s[:, 0:N], start=True, stop=True)
                    AUx = wp.tile([128, 512], BF16, tag="AUx")
                    nc.scalar.activation(
                        AUx[:, 0:N], xp[:, 0:N],
                        mybir.ActivationFunctionType.Relu, bias=c_bgv2[:],
                    )
                    a1p = pm.tile([128, 512], F32, tag="mlpp")
                    nc.tensor.matmul(a1p[:, 0:N], c_at1[:], AUx[:, 0:N], start=True, stop=False)
                    nc.tensor.matmul(a1p[:, 0:N], Qb[:], StE[:, 0:N], start=False, stop=False)
                    nc.tensor.matmul(a1p[:, 0:N], Qb2[:], StO[:, 0:N], start=False, stop=True)
                    a1s = wp.tile([128, 512], BF16, tag="a1s")
                    nc.scalar.activation(
                        a1s[:, 0:N], a1p[:, 0:N], mybir.ActivationFunctionType.Relu,
                    )
                    a2p = pm.tile([128, 512], F32, tag="mlpp")
                    nc.tensor.matmul(a2p[:, 0:N], c_at2[:], a1s[:, 0:N], start=True, stop=True)
                    a2s = wp.tile([128, 512], BF16, tag="a2s")
                    nc.scalar.activation(
                        a2s[:, 0:N], a2p[:, 0:N],
                        mybir.ActivationFunctionType.Relu, bias=c_bat2[:],
                    )

                    wlP = pw.tile([128, 8], F32, tag="wlP")
                    xtP = pt.tile([128, 512], BF16, tag="xtP")
                    for k in range(gw):
                        ks = slice(k * 128, (k + 1) * 128)
                        nc.tensor.matmul(
                            wlP[:, 2 * k : 2 * k + 2], a2s[:, ks], c_w3[:],
                            start=True, stop=True,
                        )
                        nc.tensor.transpose(xtP[:, ks], AUx[:, ks], id_b[:])

                    Pg = sp.tile([128, 8], BF16, tag="Pg")
                    nc.scalar.activation(
                        Pg[:, 0 : 2 * gw], wlP[:, 0 : 2 * gw],
                        mybir.ActivationFunctionType.Exp, bias=c_b3[:],
                    )

                    rs = sp.tile([128, 4 * 130], BF16, tag="rs")
                    nc.vector.tensor_tensor(
                        rs[:, 0 : gw * 130].rearrange("p (k h c) -> p k h c", h=2, c=65)[:, :, :, 0:64],
                        xtP[:, 0:N].rearrange("p (k h c) -> p k h c", h=2, c=64),
                        Pg[:, 0 : 2 * gw].rearrange("p (k h) -> p k h", h=2)
                        .to_broadcast([128, gw, 2, 64]),
                        mybir.AluOpType.mult,
                    )
                    nc.vector.tensor_copy(
                        rs[:, 0 : gw * 130].rearrange("p (k h c) -> p k h c", h=2, c=65)[:, :, :, 64:65],
                        Pg[:, 0 : 2 * gw].rearrange("p (k h a) -> p k h a", h=2, a=1),
                    )

                    Se = sp.tile([128, 512], BF16, tag="Se")
                    nc.vector.tensor_tensor(
                        Se[:, 0:N], c_io4[:, 0:N],
                        t_rl[:, g0 : g0 + gw].to_broadcast([128, gw, 128]),
                        mybir.AluOpType.is_equal,
                    )
                    So = sp.tile([128, 512], BF16, tag="So")
                    nc.vector.tensor_tensor(
                        So[:, 0:N], c_io4[:, 0:N],
                        t_rl[:, HNT + g0 : HNT + g0 + gw].to_broadcast([128, gw, 128]),
                        mybir.AluOpType.is_equal,
                    )

                    for k in range(gw):
                        ks = slice(k * 128, (k + 1) * 128)
                        nc.tensor.matmul(
                            acc[:, 0:65], Se[:, ks], rs[:, k * 130 : k * 130 + 65],
                            start=(isc == 0), stop=False,
                        )
                        isc += 1
                        nc.tensor.matmul(
                            acc[:, 0:65], So[:, ks], rs[:, k * 130 + 65 : k * 130 + 130],
                            start=False, stop=(isc == nsc - 1),
                        )
                        isc += 1

                # block finalize
                s_eps = fp.tile([128, 1], F32, tag="s_eps")
                nc.vector.tensor_scalar_add(s_eps[:], acc[:, 64:65], 1e-30)
                rcp = fp.tile([128, 1], F32, tag="rcp")
                nc.vector.reciprocal(rcp[:], s_eps[:])
                hn = fp.tile([128, D], BF16, tag="hn")
                nc.vector.tensor_tensor(
                    hn[:], acc[:, 0:64], rcp[:].to_broadcast([128, D]),
                    mybir.AluOpType.mult,
                )
                xtF = pt.tile([128, 512], BF16, tag="xtP")
                nc.tensor.transpose(xtF[0:64, 0:128], hn[:], id_b[:])
                hts = fp.tile([D, 128], BF16, tag="hts")
                nc.vector.tensor_copy(hts[:], xtF[0:64, 0:128])
                nc.tensor.matmul(
                    acc[:, 256:320], hts[:], c_wout[:], start=True, stop=True
                )
                outs = fp.tile([128, D], F32, tag="outs")
                nc.vector.tensor_tensor(
                    outs[:], acc[:, 256:320], c_wb[:], mybir.AluOpType.add
                )
                nc.sync.dma_start(out[b * 128 : (b + 1) * 128, :], outs[:])

    nc.compile()
    return nc


def kernel(**inputs):
    rowi = np.asarray(inputs["row_idxs"])
    coli = np.asarray(inputs["col_idxs"])
    rati = np.asarray(inputs["rating"])
    HNT, shards = _host_shard(rowi, coli, rati)

    nc = _build_program(HNT)

    def f32(x):
        return np.ascontiguousarray(np.asarray(x, dtype=np.float32))

    item_feat = f32(inputs["item_feat"])
    user_feat = f32(inputs["user_feat"])
    rating_feat = f32(inputs["rating_feat"])
    gv_w1 = f32(inputs["gv_w1"])
    att_w1 = f32(inputs["att_w1"])

    # pair-item table: h1 item contribution
    h1a = item_feat @ gv_w1[:D]                      # [I, 64]
    p2 = np.zeros((I // 2, 128), dtype=NB)
    p2[:, 0:64] = h1a[0::2]
    p2[:, 64:128] = h1a[1::2]

    T5 = rating_feat @ gv_w1[D:]                     # [5, 64]
    I64 = np.eye(64, dtype=np.float32)
    lE = np.zeros((128, 128), dtype=NB)
    lE[0:64, 0:64] = I64
    lE[64:69, 0:64] = T5
    lO = np.zeros((128, 128), dtype=NB)
    lO[64:128, 64:128] = I64
    lO[0:5, 64:128] = T5

    def bd(w):
        m = np.zeros((128, 128), dtype=NB)
        m[0:64, 0:64] = w
        m[64:128, 64:128] = w
        return m

    w3 = f32(inputs["att_w3"]).reshape(64, 1)
    w3bd = np.zeros((128, 2), dtype=NB)
    w3bd[0:64, 0:1] = w3
    w3bd[64:128, 1:2] = w3
    idt = np.zeros((64, 128), dtype=NB)
    idt[:, 0:64] = I64
    idb = np.zeros((64, 128), dtype=NB)
    idb[:, 64:128] = I64

    def lane2(v):
        return np.concatenate([v.reshape(-1), v.reshape(-1)]).reshape(128, 1).astype(np.float32)

    common = dict(
        p2_tab=p2,
        idxP=None, idxQ=None, rl=None, orTe=None, orTo=None, q1_tab=None,
        lhsE2=lE, lhsO2=lO,
        wgv2bd=bd(f32(inputs["gv_w2"])),
        wat1bd=bd(att_w1[:D]),
        wat2bd=bd(f32(inputs["att_w2"])),
        w3bd=w3bd, idtop=idt, idbot=idb,
        bgv1=lane2(f32(inputs["gv_b1"])),
        bgv2=lane2(f32(inputs["gv_b2"])),
        bat2=lane2(f32(inputs["att_b2"])),
        b3c=np.full((128, 1), np.float32(np.asarray(inputs["att_b3"]).reshape(-1)[0]),
                    dtype=np.float32),
        wout=f32(inputs["w_w"]).astype(NB),
        wb_t=np.tile(f32(inputs["w_b"]).reshape(1, D), (128, 1)),
        iota_r=np.tile(np.arange(128, dtype=np.float32), (128, 1)),
        ones_r=np.ones((1, 128), dtype=NB),
        iotap=np.arange(128, dtype=np.float32).reshape(128, 1),
        iota4=np.tile(np.arange(128, dtype=np.float32), (128, 4)),
    )

    att_b1 = f32(inputs["att_b1"]).reshape(1, D)
    in_maps = []
    for c in range(NCORES):
        base = c * UPC
        q1 = np.zeros((QROWS, 128), dtype=NB)
        q1[0:UPC, 0:64] = user_feat[base : base + UPC] @ att_w1[D:] + att_b1
        m = dict(common)
        m["q1_tab"] = q1
        q2 = np.zeros((QROWS, 128), dtype=NB)
        q2[:, 64:128] = q1[:, 0:64]
        m["q2_tab"] = q2
        m["idxP"] = shards[c]["idxP"]
        m["rlT"] = shards[c]["rlT"]
        m["rl"] = shards[c]["rl"]
        m["orTe"] = shards[c]["orTe"]
        m["orTo"] = shards[c]["orTo"]
        in_maps.append(m)

    trace = os.environ.get("ITEMAGG_TRACE") == "1"
    res = run_bass_kernel_spmd(nc, in_maps, list(range(NCORES)), trace=trace)
    global LAST_RESULT
    LAST_RESULT = res
    outs = [res.results[c]["out"][:UPC] for c in range(NCORES)]
    return np.concatenate(outs, axis=0).astype(np.float32)


LAST_RESULT = None

if __name__ == "__main__":
    pass
